# revision 14
# baseline (speedup 1.0000x reference)
"""Trainium2 Bass kernel for KnowledgeEmbeddings (ragged_sequence).

Contract: kernel(**inputs) takes FULL unsharded inputs (numpy), returns the
FULL [64, 320, 768] f32 output.  Internally shards batch rows over 8
NeuronCores (8 rows each), replicates embedding tables, and runs a Tile/Bass
kernel per core via run_bass_kernel_spmd.

V4: single fused bf16 table (word_emb ++ tt/pos ++ tt/pos+ke_b = 32570 rows,
fits int16 dma_gather indices); one 1024-row InstDMAGatherAnt per 4-tile
block covers both the word and side-table rows (SWDGE descriptor-gen on the
Q7s is ~7ns/row and was the serial bottleneck for per-tile gathers); DVE ops
restricted to fast-mode forms (tensor_tensor 2x, tensor_scalar 4x); LN sums
ride ACT accum passes; knowledge-side sums ride the psum-add via
tensor_tensor_reduce; bf16 output upcast on host.
"""

import functools
import numpy as np
import ml_dtypes

import concourse.bass as bass
import concourse.tile as tile
from concourse import bacc, mybir
from concourse.bass import IndirectOffsetOnAxis
from concourse.bass_utils import run_bass_kernel_spmd
from concourse.masks import make_identity

# Problem constants (hardcoded per spec nn_KnowledgeEmbeddings_80839874445880)
WORD_LEN = 256
KN_LEN = 64
VOCAB = 30522
N_ENT = 500000
HID = 768
MAX_POS = 512
N_TYPES = 2
D_ENT = 100
B = 64
SEQ = WORD_LEN + KN_LEN  # 320
EPS = 1e-12

NCORES = 8
ROWS = B // NCORES           # 8 batch rows per core
WT = ROWS * WORD_LEN // 128  # 16 word tiles per core
KT = ROWS * KN_LEN // 128    # 4 knowledge tiles per core
GRP = 4                      # tiles per gather block
NBLK = WT // GRP             # 4 word blocks
TBL0 = VOCAB                 # word-side tt/pos rows at wtab[TBL0:]
KTB0 = VOCAB + N_TYPES * MAX_POS   # knowledge tt/pos(+ke_b) rows
NWTAB = KTB0 + N_TYPES * MAX_POS   # 32570 <= int16 max 32767
# idx16 column layout (each gather's indices wrapped 16-way):
# [0:64) word blocks (64 cols = 4 blocks x 1024/16), [64:96) knowledge table
NI16 = NBLK * (2 * GRP * 128 // 16) + KT * 128 // 16

f32 = mybir.dt.float32
bf16 = mybir.dt.bfloat16
i32 = mybir.dt.int32
i16 = mybir.dt.int16
AF = mybir.ActivationFunctionType
ALU = mybir.AluOpType
BF16 = ml_dtypes.bfloat16


# ---------------------------------------------------------------- host side

def _compact(ids: np.ndarray, tts: np.ndarray):
    """Vectorized numpy mirror of reference._compact_row."""
    ids = ids.astype(np.int64)
    wmask = (ids > 0) & (ids < VOCAB)
    worder = np.argsort(~wmask, axis=1, kind="stable")[:, :WORD_LEN]
    nw = wmask.sum(1, keepdims=True)
    wvalid = np.arange(WORD_LEN)[None, :] < nw
    wid = np.where(wvalid, np.take_along_axis(ids, worder, 1), 0)
    wtt = np.where(wvalid, np.take_along_axis(tts, worder, 1), 1)
    wpos = np.where(wvalid, worder, np.arange(WORD_LEN)[None, :])

    kmask = ids >= VOCAB
    korder = np.argsort(~kmask, axis=1, kind="stable")[:, :KN_LEN]
    nk = kmask.sum(1, keepdims=True)
    kvalid = np.arange(KN_LEN)[None, :] < nk
    kid = np.where(kvalid, np.take_along_axis(ids, korder, 1) - VOCAB, 0)
    ktt = np.where(kvalid, np.take_along_axis(tts, korder, 1), 0)
    kpos = np.where(kvalid, korder, 0)
    return wid, wtt, wpos, kid, ktt, kpos, kvalid


def _wrap16(flat):
    """Flat gather order j -> [128, n/16] i16 tile: j at [j%16, j//16],
    replicated into all 8 groups of 16 partitions."""
    n = flat.size
    blk = flat.reshape(n // 16, 16).T.astype(np.int16)
    return np.tile(blk, (8, 1))


# ------------------------------------------------------------- device side

def _finish_stats(nc, pools, SS, SM, n, kv=None):
    """Batched [128, n] stat math.  Returns (U, RSTD) f32 column tiles."""
    spool = pools["small"]
    U_t = spool.tile([128, GRP], f32, tag="U")
    U = U_t[:, :n]
    nc.scalar.mul(U, SM, 1.0 / HID)
    SSs_t = spool.tile([128, GRP], f32, tag="SSs")
    SSs = SSs_t[:, :n]
    nc.scalar.mul(SSs, SS, 1.0 / HID)
    USQ_t = spool.tile([128, GRP], f32, tag="USQ")
    USQ = USQ_t[:, :n]
    nc.vector.tensor_mul(USQ, U, U)
    VAR_t = spool.tile([128, GRP], f32, tag="VAR")
    VAR = VAR_t[:, :n]
    nc.vector.tensor_tensor(out=VAR, in0=SSs, in1=USQ, op=ALU.subtract)
    RSTD_t = spool.tile([128, GRP], f32, tag="RSTD")
    RSTD = RSTD_t[:, :n]
    nc.scalar.activation(RSTD, VAR, func=AF.Sqrt, bias=pools["eps"][:])
    nc.vector.reciprocal(RSTD, RSTD)
    if kv is not None:
        nc.vector.tensor_mul(RSTD, RSTD, kv)
    return U, RSTD


def _device_kernel(tc, aps):
    nc = tc.nc
    wtab, ev, kwT, gb_d, idxk, idx16, kvf, out = (
        aps["wtab"], aps["entity_vec"], aps["ke_wT"], aps["gb"],
        aps["idxk"], aps["idx16"], aps["kvalid"], aps["out"],
    )
    import contextlib
    with contextlib.ExitStack() as ctx:
        singles = ctx.enter_context(tc.tile_pool(name="singles", bufs=1))
        xpool = ctx.enter_context(tc.tile_pool(name="x", bufs=3))
        spool = ctx.enter_context(tc.tile_pool(name="small", bufs=3))
        scrpool = ctx.enter_context(tc.tile_pool(name="scr", bufs=2))
        psum = ctx.enter_context(tc.tile_pool(name="psum", bufs=2, space="PSUM"))

        eps_sb = singles.tile([128, 1], f32)
        nc.vector.memset(eps_sb[:], EPS)
        pools = {"small": spool, "eps": eps_sb}

        # --- setup loads (HWDGE via sync; no Pool time) ---
        idx16_sb = singles.tile([128, NI16], i16)
        nc.sync.dma_start(idx16_sb[:], idx16)
        idxk_sb = singles.tile([128, KT], i32)
        nc.sync.dma_start(idxk_sb[:], idxk)
        kv_sb = singles.tile([128, KT], f32)
        nc.sync.dma_start(kv_sb[:], kvf)
        kw_sb = singles.tile([128, HID], f32)
        nc.vector.memset(kw_sb[:], 0.0)
        nc.sync.dma_start(kw_sb[:D_ENT, :], kwT)

        XABs = {}

        def word_gather_block(g):
            # one InstDMAGatherAnt: cols 0..3 word rows, cols 4..7 table rows
            XAB = xpool.tile([128, 2 * GRP, HID], bf16, tag="XAB")
            nc.gpsimd.dma_gather(
                out_ap=XAB[:, :, :], in_ap=wtab,
                idxs_ap=idx16_sb[:, g * 64:(g + 1) * 64],
                num_idxs=2 * GRP * 128, num_idxs_reg=2 * GRP * 128,
                elem_size=HID,
            )
            XABs[g] = XAB

        def word_compute_block(g):
            XAB = XABs[g]
            SS = spool.tile([128, GRP], f32, tag="SS")
            SM = spool.tile([128, GRP], f32, tag="SM")
            for i in range(GRP):
                X = XAB[:, i, :]
                # X += table row (2x tensor_tensor)
                nc.vector.tensor_tensor(out=X, in0=X, in1=XAB[:, GRP + i, :],
                                        op=ALU.add)
                # LN sums via two ACT accumulation passes
                scr = scrpool.tile([128, HID], bf16, tag="scr")
                nc.scalar.activation(scr[:], X, func=AF.Copy,
                                     accum_out=SM[:, i:i + 1])
                scr2 = scrpool.tile([128, HID], bf16, tag="scr")
                nc.scalar.activation(scr2[:], X, func=AF.Square,
                                     accum_out=SS[:, i:i + 1])
            U, RSTD = _finish_stats(nc, pools, SS[:], SM[:], GRP)
            for i in range(GRP):
                X = XAB[:, i, :]
                Y = XAB[:, GRP + i, :]
                # (x-u)*rstd on the 4x tensor_scalar path, then gamma/beta 2x
                nc.vector.tensor_scalar(
                    out=Y, in0=X, scalar1=U[:, i:i + 1],
                    scalar2=RSTD[:, i:i + 1],
                    op0=ALU.subtract, op1=ALU.mult,
                )
                nc.vector.tensor_tensor(out=Y, in0=Y, in1=gbb[:, 0, :],
                                        op=ALU.mult)
                nc.vector.tensor_tensor(out=Y, in0=Y, in1=gbb[:, 1, :],
                                        op=ALU.add)
                t = g * GRP + i
                b, h = divmod(t, 2)
                r = b * SEQ + h * 128
                nc.sync.dma_start(out[r:r + 128, :], Y)

        # --- Pool-queue front: the big gathers ---
        word_gather_block(0)
        word_gather_block(1)
        # knowledge gathers
        EVB = singles.tile([128, KT, D_ENT], f32)
        for c in range(KT):
            nc.gpsimd.indirect_dma_start(
                out=EVB[:, c, :], out_offset=None, in_=ev,
                in_offset=IndirectOffsetOnAxis(ap=idxk_sb[:, c:c + 1], axis=0),
            )
        XKB = singles.tile([128, KT, HID], bf16)
        nc.gpsimd.dma_gather(
            out_ap=XKB[:, :, :], in_ap=wtab,
            idxs_ap=idx16_sb[:, NBLK * 64:NBLK * 64 + 32],
            num_idxs=KT * 128, num_idxs_reg=KT * 128, elem_size=HID,
        )
        word_gather_block(2)

        # broadcasts + identity (Pool engine, small)
        ident = singles.tile([128, 128], f32)
        make_identity(nc, ident[:])
        gbb = singles.tile([128, 4, HID], bf16)
        gb_bcast = bass.AP(tensor=gb_d.tensor, offset=gb_d.offset,
                           ap=[[0, 128]] + list(gb_d.ap))
        nc.gpsimd.dma_start(out=gbb[:], in_=gb_bcast)

        # knowledge PE prep: transpose ev rows, pad, matmul
        EVTs = []
        for c in range(KT):
            ps_t = psum.tile([D_ENT, 128], f32, tag="pst")
            nc.tensor.transpose(out=ps_t[:], in_=EVB[:, c, :],
                                identity=ident[:])
            EVT = singles.tile([128, 128], f32, name=f"EVT{c}")
            nc.vector.memset(EVT[96:, :], 0.0)
            nc.scalar.copy(EVT[:D_ENT, :], ps_t[:])
            EVTs.append(EVT)
        PMs = []
        for c in range(KT):
            for half in range(2):
                pm = psum.tile([128, 384], f32, tag="mm", bufs=4)
                nc.tensor.matmul(
                    out=pm[:], lhsT=EVTs[c][:],
                    rhs=kw_sb[:, 384 * half:384 * (half + 1)],
                    start=True, stop=True,
                )
                PMs.append(pm)

        word_compute_block(0)
        word_gather_block(3)
        word_compute_block(1)

        # knowledge adds: XK = psum + table rows
        XK = singles.tile([128, KT, HID], bf16)
        SMK = singles.tile([128, KT], f32)
        SSK = singles.tile([128, KT], f32)
        for c in range(KT):
            for half in range(2):
                sl = slice(384 * half, 384 * (half + 1))
                nc.vector.tensor_tensor(
                    out=XK[:, c, sl], in0=PMs[2 * c + half][:],
                    in1=XKB[:, c, sl], op=ALU.add,
                )
            scrk = scrpool.tile([128, HID], bf16, tag="scr")
            nc.scalar.activation(scrk[:], XK[:, c, :], func=AF.Copy,
                                 accum_out=SMK[:, c:c + 1])
            scrk2 = scrpool.tile([128, HID], bf16, tag="scr")
            nc.scalar.activation(scrk2[:], XK[:, c, :], func=AF.Square,
                                 accum_out=SSK[:, c:c + 1])

        word_compute_block(2)
        word_compute_block(3)

        # --- knowledge tail ---
        U, RSTD = _finish_stats(nc, pools, SSK[:], SMK[:], KT, kv=kv_sb[:])
        for c in range(KT):
            Y = XKB[:, c, :]
            nc.vector.tensor_scalar(
                out=Y, in0=XK[:, c, :], scalar1=U[:, c:c + 1],
                scalar2=RSTD[:, c:c + 1], op0=ALU.subtract, op1=ALU.mult,
            )
            nc.vector.tensor_tensor(out=Y, in0=Y, in1=gbb[:, 2, :],
                                    op=ALU.mult)
            nc.vector.tensor_tensor(out=Y, in0=Y, in1=gbb[:, 3, :],
                                    op=ALU.add)
            r0 = (2 * c) * SEQ + WORD_LEN
            r1 = (2 * c + 1) * SEQ + WORD_LEN
            nc.sync.dma_start(out[r0:r0 + 64, :], Y[0:64, :])
            nc.sync.dma_start(out[r1:r1 + 64, :], Y[64:128, :])


@functools.lru_cache(maxsize=1)
def build_program():
    nc = bacc.Bacc("TRN2", target_bir_lowering=False, debug=False,
                   enable_asserts=False)
    aps = {
        "wtab": nc.dram_tensor("wtab", [NWTAB, HID], bf16,
                               kind="ExternalInput").ap(),
        "entity_vec": nc.dram_tensor("entity_vec", [N_ENT, D_ENT], f32,
                                     kind="ExternalInput").ap(),
        "ke_wT": nc.dram_tensor("ke_wT", [D_ENT, HID], f32,
                                kind="ExternalInput").ap(),
        "gb": nc.dram_tensor("gb", [4, HID], bf16,
                             kind="ExternalInput").ap(),
        "idxk": nc.dram_tensor("idxk", [128, KT], i32,
                               kind="ExternalInput").ap(),
        "idx16": nc.dram_tensor("idx16", [128, NI16], i16,
                                kind="ExternalInput").ap(),
        "kvalid": nc.dram_tensor("kvalid", [128, KT], f32,
                                 kind="ExternalInput").ap(),
        "out": nc.dram_tensor("out", [ROWS * SEQ, HID], bf16,
                              kind="ExternalOutput").ap(),
    }
    with tile.TileContext(nc) as tc:
        _device_kernel(tc, aps)
    nc.compile()
    return nc


def _prepare_in_maps(inputs):
    input_ids = np.asarray(inputs["input_ids"], dtype=np.int32)
    token_type_ids = np.asarray(inputs["token_type_ids"], dtype=np.int32)
    word_emb = np.asarray(inputs["word_emb"], np.float32)
    pos_emb = np.asarray(inputs["pos_emb"], np.float32)
    tt_emb = np.asarray(inputs["tt_emb"], np.float32)
    entity_vec = np.ascontiguousarray(np.asarray(inputs["entityVec"], np.float32))
    ke_w = np.asarray(inputs["ke_w"], np.float32)
    ke_b = np.asarray(inputs["ke_b"], np.float32)

    # fused table: [word_emb; tt/pos rows; tt/pos + ke_b rows], all bf16
    base = (tt_emb[:, None, :] + pos_emb[None, :, :]).reshape(
        N_TYPES * MAX_POS, HID)
    wtab = np.ascontiguousarray(np.concatenate(
        [word_emb, base, base + ke_b[None, :]], axis=0).astype(BF16))
    ke_wT = np.ascontiguousarray(ke_w.T)
    gb = np.ascontiguousarray(np.stack([
        np.asarray(inputs["w_gamma"], np.float32),
        np.asarray(inputs["w_beta"], np.float32),
        np.asarray(inputs["k_gamma"], np.float32),
        np.asarray(inputs["k_beta"], np.float32),
    ]).astype(BF16))

    wid, wtt, wpos, kid, ktt, kpos, kvalid = _compact(input_ids, token_type_ids)
    widx = wid.astype(np.int32)
    wtidx = (TBL0 + wpos + MAX_POS * wtt).astype(np.int32)
    kidx = kid.astype(np.int32)
    ktidx = (KTB0 + kpos + MAX_POS * ktt).astype(np.int32)
    kvf = kvalid.astype(np.float32)

    in_maps = []
    for c in range(NCORES):
        s = slice(c * ROWS, (c + 1) * ROWS)
        wflat = widx[s].reshape(WT, 128)     # [tile, p]
        tflat = wtidx[s].reshape(WT, 128)
        kflat = ktidx[s].reshape(KT, 128)
        cols = []
        for g in range(NBLK):
            blk = np.concatenate([wflat[g * GRP:(g + 1) * GRP],
                                  tflat[g * GRP:(g + 1) * GRP]]).reshape(-1)
            cols.append(_wrap16(blk))
        cols.append(_wrap16(kflat.reshape(-1)))
        idx16 = np.concatenate(cols, axis=1)
        in_maps.append({
            "wtab": wtab,
            "entity_vec": entity_vec,
            "ke_wT": ke_wT,
            "gb": gb,
            "idxk": np.ascontiguousarray(kidx[s].reshape(KT, 128).T),
            "idx16": np.ascontiguousarray(idx16),
            "kvalid": np.ascontiguousarray(kvf[s].reshape(KT, 128).T),
        })
    return in_maps


def run(inputs, trace=False):
    """Returns (full_output [64,320,768] f32, exec_time_ns or None)."""
    nc = build_program()
    in_maps = _prepare_in_maps(inputs)
    res = run_bass_kernel_spmd(nc, in_maps, list(range(NCORES)), trace=trace)
    out = np.concatenate(
        [np.asarray(r["out"]).astype(np.float32).reshape(ROWS, SEQ, HID)
         for r in res.results], axis=0)
    return out, res.exec_time_ns


def kernel(**inputs) -> np.ndarray:
    out, _ = run(inputs)
    return out
